# revision 30
# baseline (speedup 1.0000x reference)
"""DeepseekV3 attention (B=1, S=2048, D=2048, H=16, KV=4) on 8 trn2 cores.

Sharding: token-modulo-8 split. Core c owns query tokens {t : t % 8 == c}
(256 each) — causal attention work is identical on every core, so one SPMD
program serves all 8 cores with per-core DATA (host-sliced hidden columns,
cos/sin slices, causal band masks) carrying the differences.

v2: all matmul operands in bf16 (fp32 PSUM accumulation), head-PAIR packed
attention so most matmuls stream 512 columns, softmax sum/AV matmuls
software-pipelined 3 iterations behind scores/exp so the tensor queue never
waits on the scalar engine, causal masks added on the (otherwise idle)
gpsimd engine, reciprocal/normalize on vector.

Per core:
  - kv path replicated: ckv^T = wkv_a^T @ h^T over all 2048 tokens, RMS,
    k_nope^T / v via wkv_b, RoPE on k_rot.
  - q path token-split: q_a^T/q^T only for the core's 256 tokens; q tiles
    packed per head pair: qn2[p] = [128, 2*256], qr2[p] = [64, 2*256].
  - attention: scores computed transposed [k, q] (lhsT = k^T tiles), softmax
    denominator via ones-matmul, AV with v in natural [token, dv] layout.
    Causal masking: additive band masks (input data) on the diagonal bands.
  - o_proj over the core's 256 token rows; host reassembles rows.
"""
import math
import sys
import types

import numpy as np
import ml_dtypes

# ---------------------------------------------------------------------------
# Container compat: this walrus build rejects instructions carrying more than
# one sync-wait command. Patch Tile to (a) split multi-wait instructions into
# single-wait NoOps on the same engine, (b) hoist the end-of-kernel drain's
# waits onto single-wait NOPs. Also register the NTFF profile hook (the
# image's antenv lacks axon_hooks) so trace=True works for profiling.
# ---------------------------------------------------------------------------
import concourse.bass as bass
import concourse.mybir as mybir
import concourse.tile as tile
from concourse.bass_utils import run_bass_kernel_spmd
from concourse.tile import ScopedClock
from bass_rust import VectorClock

N_PROCS = len(VectorClock())
_PATCHED = False


def _install_ntff_hook():
    if 'antenv.axon_hooks' in sys.modules:
        return
    m = types.ModuleType('antenv.axon_hooks')
    holder = [None]
    m.set_axon_ntff_profile_hook = lambda h: holder.__setitem__(0, h)
    m.get_axon_ntff_profile_hook = lambda: holder[0]
    sys.modules['antenv.axon_hooks'] = m
    try:
        from trn_agent_boot.trn_boot import _ntff_profile_via_ctypes
        m.set_axon_ntff_profile_hook(
            _ntff_profile_via_ctypes('/opt/axon/libaxon_pjrt.so'))
    except Exception:
        pass


def _patched_drain_and_barrier(self, tick_clock, wait_clock):
    gc = tick_clock.global_clock
    for p in range(N_PROCS):
        if gc[p] == 0:
            continue
        single = VectorClock([gc[q] if q == p else 0 for q in range(N_PROCS)])
        nop_inst = self.nc.sync.nop(nofuse=True)
        wait_clock.add_sem_waits(nop_inst.ins, ScopedClock({None: single}))
    self.nc.sync.drain()
    self.nc.all_engine_barrier()
    popped = self.nc._tile_sem_poison_stack.pop()
    assert popped is self._sem_poison
    self.nc.clear_and_free_semaphores(list(self.sems.allocated().values()))
    self.nc.all_engine_barrier()


def _make_split_lower(orig):
    def _split_multi_waits(self, ordered):
        nc = self.nc
        for bb_name, insts in ordered.items():
            out = []
            for inst in insts:
                si = inst.sync_info
                waits = list(si.on_wait) if si is not None else []
                if len(waits) > 1:
                    for w in waits[:-1]:
                        nop = mybir.InstNoOp(
                            name=f"{inst.name}-waitsplit-{nc.next_id()}",
                            engine=inst.engine,
                            sync_info=mybir.SyncInfo(on_wait=[w], on_update=[]),
                        )
                        nc.register_instruction(nop)
                        out.append(nop)
                    inst.sync_info = mybir.SyncInfo(
                        on_wait=[waits[-1]], on_update=list(si.on_update))
                out.append(inst)
            ordered[bb_name] = out
        return orig(self, ordered)
    return _split_multi_waits


def _install_patches():
    global _PATCHED
    _install_ntff_hook()
    if _PATCHED:
        return
    tile.TileContext._drain_and_barrier = _patched_drain_and_barrier
    tile.TileContext._lower_ordered_insts = _make_split_lower(
        tile.TileContext._lower_ordered_insts)
    _PATCHED = True


_install_patches()

# ---------------------------------------------------------------------------
# Problem constants (hardcoded per the spec).
# ---------------------------------------------------------------------------
S = 2048
D = 2048
H = 16
KV = 4
GROUPS = H // KV
DN = 128          # d_nope
DR = 64           # d_rope
DQK = DN + DR     # 192
DV = 128
QR = 1536         # q rank
KVR = 512         # kv rank
EPS = 1e-6
NC_ = 8           # cores
TPC = S // NC_    # 256 tokens per core
NB = S // 128     # 16 k-subtiles
NPAIR = H // 2    # 8 head pairs
SCALE = 1.0 / math.sqrt(DQK)
NEG = -1e30
PIPE = 5          # sum/AV matmuls trail scores/exp by this many k-blocks

F32 = mybir.dt.float32
BF16 = mybir.dt.bfloat16
BF = ml_dtypes.bfloat16

_BUILT = None     # cached (nc,) so repeat kernel() calls skip rebuild
LAST_RESULTS = None  # BassKernelResults stash for test.py


def _build():
    nc = bass.Bass()

    # ---- DRAM I/O (identical declaration on all cores; data differs) ----
    hT = nc.dram_tensor("hT", [D, S], BF16, kind="ExternalInput")
    hTq = nc.dram_tensor("hTq", [D, TPC], BF16, kind="ExternalInput")
    wqa = nc.dram_tensor("wqa", [D, QR], BF16, kind="ExternalInput")
    wqbn = nc.dram_tensor("wqbn", [QR, H * DN], BF16, kind="ExternalInput")
    wqbr = nc.dram_tensor("wqbr", [QR, H * DR], BF16, kind="ExternalInput")
    wkva = nc.dram_tensor("wkva", [D, KVR + DR], BF16, kind="ExternalInput")
    wkvbk = nc.dram_tensor("wkvbk", [KVR, KV * DN], BF16, kind="ExternalInput")
    wkvbv = nc.dram_tensor("wkvbv", [KVR, KV * DV], BF16, kind="ExternalInput")
    wo_t = nc.dram_tensor("wo", [H * DV, D], BF16, kind="ExternalInput")
    cossinT = nc.dram_tensor("cossinT", [2 * DR, S], F32, kind="ExternalInput")
    cosq2 = nc.dram_tensor("cosq2", [2 * DR, TPC], F32, kind="ExternalInput")
    sinq2 = nc.dram_tensor("sinq2", [2 * DR, TPC], F32, kind="ExternalInput")
    # causal band mask: band[k, j] = 0 if k <= 8j + c else NEG. Within key
    # block kb the causal boundary only spans q columns [16kb, 16kb+16) and
    # the band contents are kb-independent, so one [128, 16] tile serves
    # every block.
    band = nc.dram_tensor("band", [128, 16], F32, kind="ExternalInput")
    out = nc.dram_tensor("out", [TPC, D], F32, kind="ExternalOutput")
    # scratch for free->partition broadcasts
    scr_k = nc.dram_tensor("scr_k", [1, S], F32, kind="Internal")
    scr_q = nc.dram_tensor("scr_q", [1, TPC], F32, kind="Internal")
    scr_r = nc.dram_tensor("scr_r", [NPAIR, 512], F32, kind="Internal")

    def bcast_src(dram, off, ncols):
        # element off.. of the flat DRAM vector, broadcast to 128 partitions
        ap = dram[:]
        return bass.AP(tensor=ap.tensor, offset=ap.offset + off,
                       ap=[[0, 128], [1, ncols]])

    with tile.TileContext(nc) as tc:
        with (
            tc.tile_pool(name="persist", bufs=1) as P,   # attention-lived
            tc.tile_pool(name="wstream", bufs=3) as WS,  # streamed weights
            tc.tile_pool(name="ppool", bufs=6) as PP,    # p tiles (bf16)
        ):
            ones_b = P.tile([128, 1], BF16, name="ones_b")
            nc.vector.memset(ones_b[:], 1.0)
            eps_sb = P.tile([1, 1], F32, name="eps_sb")
            nc.vector.memset(eps_sb[:], EPS)

            # attention-lived products
            qn2 = [P.tile([128, 512], BF16, name=f"qn2_{p}")
                   for p in range(NPAIR)]
            qr2 = [P.tile([64, 512], BF16, name=f"qr2_{p}")
                   for p in range(NPAIR)]
            knopeT = [P.tile([128, S], BF16, name=f"knopeT{h}")
                      for h in range(KV)]
            krot = P.tile([64, S], BF16, name="krot")
            v_sb = [P.tile([128, KV * DV], BF16, name=f"v{m}")
                    for m in range(16)]
            attn2 = [P.tile([128, 512], BF16, name=f"attn2_{p}")
                     for p in range(NPAIR)]

            band_sb = P.tile([128, 16], F32, name="band_sb")
            nc.gpsimd.dma_start(band_sb[:], band[:, :])

            # kv weights resident from the start (gpsimd queue, overlapping
            # the q window's compute + sync/scalar weight streams)
            wkva_sb = [P.tile([128, KVR + DR], BF16, name=f"wkva{k}")
                       for k in range(16)]
            for k in range(16):
                nc.gpsimd.dma_start(wkva_sb[k][:],
                                    wkva[k * 128:(k + 1) * 128, :])
            wkvbk_sb = [P.tile([128, KV * DN], BF16, name=f"wkvbk{k}")
                        for k in range(4)]
            wkvbv_sb = [P.tile([128, KV * DV], BF16, name=f"wkvbv{k}")
                        for k in range(4)]
            for k in range(4):
                nc.gpsimd.dma_start(wkvbk_sb[k][:],
                                    wkvbk[k * 128:(k + 1) * 128, :])
                nc.gpsimd.dma_start(wkvbv_sb[k][:],
                                    wkvbv[k * 128:(k + 1) * 128, :])

            # ========================= Q window =========================
            with (
                tc.tile_pool(name="qwin", bufs=1) as QW,
                tc.tile_pool(name="qsc", bufs=2) as QS,
            ):
                # wq_b resident: [128, 3072] per k-chunk (nope 0:2048,
                # rope 2048:3072) so the q_b loop issues no DMAs at all.
                # Scalar queue: keeps sync free for the wqa stream.
                wqb_sb = [QW.tile([128, H * DQK], BF16, name=f"wqb{k}")
                          for k in range(12)]
                for k in range(12):
                    nc.scalar.dma_start(wqb_sb[k][:, 0:H * DN],
                                        wqbn[k * 128:(k + 1) * 128, :])
                    nc.scalar.dma_start(wqb_sb[k][:, H * DN:],
                                        wqbr[k * 128:(k + 1) * 128, :])

                # q_a^T [1536, 256] bf16 (raw, pre-rms)
                qaT = [QW.tile([128, TPC], BF16, name=f"qaT{m}")
                       for m in range(12)]
                with tc.tile_pool(name="qaps", bufs=1, space="PSUM") as PSB:
                    for half in range(2):
                        pss = [PSB.tile([128, TPC], F32, name=f"ps_qa{m}",
                                        tag=f"ps_qa{m}") for m in range(6)]
                        for k in range(16):
                            wch = WS.tile([128, 768], BF16, name="wch",
                                          tag="wch", bufs=3)
                            nc.sync.dma_start(
                                wch[:], wqa[k * 128:(k + 1) * 128,
                                            half * 768:(half + 1) * 768])
                            hch = QS.tile([128, TPC], BF16, name="hqch",
                                          tag="hqch", bufs=4)
                            nc.scalar.dma_start(
                                hch[:], hTq[k * 128:(k + 1) * 128, :])
                            for m in range(6):
                                nc.tensor.matmul(
                                    pss[m][:], wch[:, m * 128:(m + 1) * 128],
                                    hch[:], start=(k == 0), stop=(k == 15))
                        for m in range(6):
                            nc.vector.tensor_copy(qaT[half * 6 + m][:],
                                                  pss[m][:])

                    # q RMS scale vector (applied at q_b evac: per-token
                    # scaling commutes through the matmul)
                    ps_qss = PSB.tile([1, TPC], F32, name="ps_qss")
                    for m in range(12):
                        sq = QS.tile([128, TPC], BF16, name="sqq", tag="sqq")
                        nc.gpsimd.tensor_mul(sq[:], qaT[m][:], qaT[m][:])
                        nc.tensor.matmul(ps_qss[:], ones_b[:], sq[:],
                                         start=(m == 0), stop=(m == 11))
                    srt_q = QW.tile([1, TPC], F32, name="srt_q")
                    nc.scalar.activation(srt_q[:], ps_qss[:],
                                         mybir.ActivationFunctionType.Sqrt,
                                         bias=eps_sb[:], scale=1.0 / QR)
                    nc.scalar.dma_start(scr_q[:], srt_q[:])
                    qsc_bc = QW.tile([128, TPC], F32, name="qsc_bc")
                    nc.scalar.dma_start(qsc_bc[:], bcast_src(scr_q, 0, TPC))
                    qsc_r = QW.tile([128, TPC], F32, name="qsc_r")
                    nc.vector.reciprocal(qsc_r[:], qsc_bc[:])

                # q_b per head pair: nope [128,256]x2 + rope pair [128,256];
                # rms scale applied at evac
                cosq_sb = QW.tile([128, TPC], F32, name="cosq_sb")
                sinq_sb = QW.tile([128, TPC], F32, name="sinq_sb")
                nc.sync.dma_start(cosq_sb[:], cosq2[:, :])
                nc.sync.dma_start(sinq_sb[:], sinq2[:, :])
                with tc.tile_pool(name="qbps", bufs=2, space="PSUM") as PSB:
                    for p in range(NPAIR):
                        pn = [PSB.tile([128, TPC], F32, name=f"ps_qn{e}",
                                       tag=f"ps_qn{e}") for e in range(2)]
                        pr = PSB.tile([128, TPC], F32, name="ps_qr",
                                      tag="ps_qr")
                        for k in range(12):
                            for e in range(2):
                                nc.tensor.matmul(
                                    pn[e][:],
                                    wqb_sb[k][:, p * 256 + e * 128:
                                              p * 256 + (e + 1) * 128],
                                    qaT[k][:], start=(k == 0), stop=(k == 11))
                            nc.tensor.matmul(
                                pr[:],
                                wqb_sb[k][:, H * DN + p * 128:
                                          H * DN + (p + 1) * 128],
                                qaT[k][:], start=(k == 0), stop=(k == 11))
                        for e in range(2):
                            nc.vector.tensor_mul(
                                qn2[p][:, e * 256:(e + 1) * 256],
                                pn[e][:], qsc_r[:])
                        # RoPE on the rope pair (rows 0-63 head 2p,
                        # 64-127 head 2p+1): out = x*cos2 + rot(x)*sin2 with
                        # rot = partition rotate by 32 within each 64-row
                        # block (via sbuf->sbuf DMA), rotate_half sign folded
                        # into sin2 host-side.
                        qrf = QS.tile([128, TPC], F32, name="qrf", tag="qrf", bufs=1)
                        nc.vector.tensor_mul(qrf[:], pr[:], qsc_r[:])
                        xr = QS.tile([128, TPC], F32, name="xr", tag="xr", bufs=1)
                        for b0, b1 in ((0, 32), (32, 0), (64, 96), (96, 64)):
                            nc.gpsimd.dma_start(xr[b0:b0 + 32, :],
                                                qrf[b1:b1 + 32, :])
                        t1 = QS.tile([128, TPC], F32, name="t1q", tag="t1q", bufs=1)
                        nc.vector.tensor_mul(t1[:], qrf[:], cosq_sb[:])
                        nc.vector.tensor_mul(xr[:], xr[:], sinq_sb[:])
                        qrx = QS.tile([128, TPC], BF16, name="qrx", tag="qrx")
                        nc.vector.tensor_add(qrx[:], t1[:], xr[:])
                        # pack pair: head 2p -> cols 0:256, head 2p+1 ->
                        # cols 256:512 (partition shift via sbuf DMA)
                        nc.vector.tensor_copy(qr2[p][0:64, 0:256],
                                              qrx[0:64, :])
                        nc.gpsimd.dma_start(qr2[p][0:64, 256:512],
                                            qrx[64:128, :])

            # ========================= KV window =========================
            # fully chunked over 4 token chunks of 512: a-proj -> rms ->
            # rope -> k_nope^T -> v, per chunk.
            with (
                tc.tile_pool(name="kvwin", bufs=1) as KW,
                tc.tile_pool(name="kvch", bufs=1) as KC,
                tc.tile_pool(name="ksc", bufs=1) as KS,
                tc.tile_pool(name="kps", bufs=1, space="PSUM") as PSB,
            ):
                m_sizes = [128, 128, 128, 128, 64]
                for n in range(4):
                    ncols = slice(n * 512, (n + 1) * 512)
                    # ---- a-projection for this chunk ----
                    ckv = [KC.tile([m_sizes[m], 512], BF16, name=f"ckv{m}",
                                   tag=f"ckv{m}") for m in range(5)]
                    pss = [PSB.tile([m_sizes[m], 512], F32, name=f"ps_kva{m}",
                                    tag=f"ps_kva{m}") for m in range(5)]
                    for k in range(16):
                        hch = WS.tile([128, 512], BF16, name="hch", tag="hch",
                                      bufs=4)
                        eng = nc.sync if k % 2 == 0 else nc.scalar
                        eng.dma_start(hch[:],
                                      hT[k * 128:(k + 1) * 128, ncols])
                        for m in range(5):
                            nc.tensor.matmul(
                                pss[m][:],
                                wkva_sb[k][:, m * 128: m * 128 + m_sizes[m]],
                                hch[:], start=(k == 0), stop=(k == 15))
                    for m in range(4):
                        nc.vector.tensor_copy(ckv[m][:], pss[m][:])

                    # ---- RoPE on k_rot (raw; no rms on the rope part):
                    # out = x*cos + rot(x)*sin_signed, rot via DMA ----
                    cos_t = KS.tile([64, 512], F32, name="cos_t", tag="cos_t",
                                    bufs=2)
                    sin_t = KS.tile([64, 512], F32, name="sin_t", tag="sin_t",
                                    bufs=2)
                    nc.gpsimd.dma_start(cos_t[:], cossinT[0:64, ncols])
                    nc.gpsimd.dma_start(sin_t[:], cossinT[64:128, ncols])
                    krr = KS.tile([64, 512], F32, name="krr", tag="krr")
                    nc.vector.tensor_copy(krr[:], pss[4][:])
                    kxr = KS.tile([64, 512], F32, name="kxr", tag="kxr")
                    nc.gpsimd.dma_start(kxr[0:32, :], krr[32:64, :])
                    nc.gpsimd.dma_start(kxr[32:64, :], krr[0:32, :])
                    kt1 = KS.tile([64, 512], F32, name="kt1", tag="kt1")
                    nc.vector.tensor_mul(kt1[:], krr[:], cos_t[:])
                    nc.vector.tensor_mul(kxr[:], kxr[:], sin_t[:])
                    nc.vector.tensor_add(krot[0:64, ncols], kt1[:], kxr[:])

                    # ---- RMS scale vector for this chunk ----
                    ps_ss = PSB.tile([1, 512], F32, name="ps_ssk",
                                     tag="ps_ssk")
                    for m in range(4):
                        sq = KS.tile([128, 512], BF16, name="sqk", tag="sqk")
                        nc.gpsimd.tensor_mul(sq[:], ckv[m][:], ckv[m][:])
                        nc.tensor.matmul(ps_ss[:], ones_b[:], sq[:],
                                         start=(m == 0), stop=(m == 3))
                    srt_k = KS.tile([1, 512], F32, name="srt_k", tag="srt_k")
                    nc.scalar.activation(srt_k[:], ps_ss[:],
                                         mybir.ActivationFunctionType.Sqrt,
                                         bias=eps_sb[:], scale=1.0 / KVR)
                    nc.scalar.dma_start(scr_k[:, ncols], srt_k[:])
                    ksc_bc = KS.tile([128, 512], F32, name="ksc_bc",
                                     tag="ksc_bc")
                    nc.gpsimd.dma_start(ksc_bc[:],
                                        bcast_src(scr_k, n * 512, 512))
                    ksc_r = KS.tile([128, 512], F32, name="ksc_r",
                                    tag="ksc_r")
                    nc.vector.reciprocal(ksc_r[:], ksc_bc[:])
                    # token-partition-shaped scale for v evac:
                    # [p, m] <-> token 128m + p of this chunk
                    kscT = KS.tile([128, 4], F32, name="kscT", tag="kscT")
                    skap = scr_k[:]
                    nc.gpsimd.dma_start(
                        kscT[:],
                        bass.AP(tensor=skap.tensor,
                                offset=skap.offset + n * 512,
                                ap=[[1, 128], [128, 4]]))
                    kscT_r = KS.tile([128, 4], F32, name="kscT_r",
                                     tag="kscT_r")
                    nc.vector.reciprocal(kscT_r[:], kscT[:])

                    # ---- k_nope^T for this chunk (rms scale at evac) ----
                    for h in range(KV):
                        ps = PSB.tile([128, 512], F32, name="ps_kn",
                                      tag="ps_kn")
                        for k in range(4):
                            nc.tensor.matmul(
                                ps[:], wkvbk_sb[k][:, h * 128:(h + 1) * 128],
                                ckv[k][:], start=(k == 0), stop=(k == 3))
                        nc.vector.tensor_mul(knopeT[h][:, ncols], ps[:],
                                             ksc_r[:])

                    # ---- v natural for this chunk's 4 token tiles ----
                    for mm in range(4):
                        ps = PSB.tile([128, 512], F32, name="ps_v", tag="ps_v")
                        for k in range(4):
                            nc.tensor.matmul(
                                ps[:], ckv[k][:, mm * 128:(mm + 1) * 128],
                                wkvbv_sb[k][:], start=(k == 0), stop=(k == 3))
                        nc.vector.tensor_scalar_mul(v_sb[n * 4 + mm][:],
                                                    ps[:],
                                                    kscT_r[:, mm:mm + 1])

            # =========================== Attention ==========================
            # Per head pair p (heads 2p, 2p+1 sharing kv head p//2):
            # scores^T [keys, 2x256 q] in PSUM via 512-col matmuls, causal
            # band mask added on gpsimd, exp on scalar -> p_t (bf16), softmax
            # denominator + AV matmuls trail by PIPE k-blocks so the tensor
            # queue never waits on exp. q-col halves that are fully causal-
            # masked (head-local cols 0:128 for kb >= 8) are skipped.
            with tc.tile_pool(name="aps", bufs=1, space="PSUM") as PSA:
                for p in range(NPAIR):
                    hk = p // 2
                    ps_av = PSA.tile([128, 512], F32, name="ps_av",
                                     tag="ps_av", bufs=2)
                    ps_sum = PSA.tile([1, 512], F32, name="ps_sum",
                                      tag="ps_sum", bufs=2)

                    def hi2(t):
                        # strided view: head-local cols 128:256 of both heads
                        return t.rearrange("p (h c) -> p h c", h=2)[:, :,
                                                                   128:256]

                    def sum_av(kb0, pt0, ps_sum=ps_sum, ps_av=ps_av, hk=hk):
                        st, sp = (kb0 == 0), (kb0 == NB - 1)
                        if kb0 < 8:
                            nc.tensor.matmul(ps_sum[:], ones_b[:], pt0[:],
                                             start=st, stop=sp,
                                             skip_group_check=True)
                            nc.tensor.matmul(
                                ps_av[:], v_sb[kb0][:, hk * 128:(hk + 1) * 128],
                                pt0[:], start=st, stop=sp,
                                skip_group_check=True)
                        else:
                            nc.tensor.matmul(hi2(ps_sum[:]), ones_b[:],
                                             hi2(pt0[:]), start=False,
                                             stop=sp, skip_group_check=True)
                            nc.tensor.matmul(
                                hi2(ps_av[:]),
                                v_sb[kb0][:, hk * 128:(hk + 1) * 128],
                                hi2(pt0[:]), start=False, stop=sp,
                                skip_group_check=True)

                    pend = []
                    for kb in range(NB):
                        kcols = slice(kb * 128, (kb + 1) * 128)
                        lo_blk = 0 if kb < 8 else 128   # matmul block start
                        lo = 16 * kb                    # first valid q col
                        ps_sc = PSA.tile([128, 512], F32, name="ps_sc",
                                         tag="ps_sc", bufs=4)
                        p_t = PP.tile([128, 512], BF16, name="p_t", tag="p_t",
                                      bufs=7)
                        if kb < 8:
                            nc.tensor.matmul(ps_sc[:], knopeT[hk][:, kcols],
                                             qn2[p][:], start=True, stop=False)
                            nc.tensor.matmul(ps_sc[:], krot[0:64, kcols],
                                             qr2[p][0:64, :], start=False,
                                             stop=True)
                        else:
                            nc.tensor.matmul(hi2(ps_sc[:]),
                                             knopeT[hk][:, kcols],
                                             hi2(qn2[p][:]), start=True,
                                             stop=False)
                            nc.tensor.matmul(hi2(ps_sc[:]),
                                             krot[0:64, kcols],
                                             hi2(qr2[p][0:64, :]),
                                             start=False, stop=True)
                        for e in range(2):
                            # causal band add (vector), zero the dead p_t
                            # columns the block matmuls will still read
                            # (gpsimd), exp only the valid columns (scalar).
                            bsl = slice(e * 256 + lo, e * 256 + lo + 16)
                            nc.vector.tensor_add(ps_sc[:, bsl], ps_sc[:, bsl],
                                                 band_sb[:, 0:16])
                            if lo > lo_blk:
                                nc.gpsimd.memset(
                                    p_t[:, e * 256 + lo_blk:e * 256 + lo],
                                    0.0)
                            vsl = slice(e * 256 + lo, e * 256 + 256)
                            nc.scalar.activation(
                                p_t[:, vsl], ps_sc[:, vsl],
                                mybir.ActivationFunctionType.Exp, scale=SCALE)
                        pend.append((kb, p_t))
                        if len(pend) > PIPE:
                            sum_av(*pend.pop(0))
                    for it in pend:
                        sum_av(*it)

                    # normalize by the softmax sums (free->partition bcast)
                    rec_t = PP.tile([1, 512], F32, name="rec_t",
                                    tag="rec_t", bufs=2)
                    nc.vector.reciprocal(rec_t[:], ps_sum[:])
                    nc.gpsimd.dma_start(scr_r[p:p + 1, :], rec_t[:])
                    rb = PP.tile([128, 512], F32, name="rb", tag="rb",
                                 bufs=2)
                    nc.gpsimd.dma_start(rb[:], bcast_src(scr_r, p * 512, 512))
                    nc.vector.tensor_mul(attn2[p][:], ps_av[:], rb[:])

            # ============================ o_proj ============================
            with (
                tc.tile_pool(name="ops", bufs=1, space="PSUM") as PSB,
                tc.tile_pool(name="ostream", bufs=3) as OS,
            ):
                # full-row wo loads (1 DMA per head) feeding all 4 D-chunks;
                # all 8 [tok-half, D-chunk] accumulators live at once.
                pso = [PSB.tile([128, 512], F32, name=f"ps_o{n}_{m}",
                                tag=f"ps_o{n}_{m}")
                       for n in range(4) for m in range(2)]
                for h in range(H):
                    p, e = h // 2, h % 2
                    wch = OS.tile([128, D], BF16, name="wcho", tag="wcho")
                    nc.sync.dma_start(wch[:],
                                      wo_t[h * 128:(h + 1) * 128, :])
                    for n in range(4):
                        for m in range(2):
                            nc.tensor.matmul(
                                pso[2 * n + m][:],
                                attn2[p][:, e * 256 + m * 128:
                                         e * 256 + (m + 1) * 128],
                                wch[:, n * 512:(n + 1) * 512],
                                start=(h == 0), stop=(h == H - 1))
                for n in range(4):
                    for m in range(2):
                        osb = PP.tile([128, 512], F32, name="osb", tag="osb",
                                      bufs=3)
                        nc.vector.tensor_copy(osb[:], pso[2 * n + m][:])
                        nc.scalar.dma_start(
                            out[m * 128:(m + 1) * 128,
                                n * 512:(n + 1) * 512], osb[:])

    return nc


def kernel(hidden_states, cos, sin, wq_a, q_a_ln_w, wq_b, wkv_a, kv_a_ln_w,
           wkv_b, wo, cache_position, _trace=False):
    global _BUILT, LAST_RESULTS
    hidden_states = np.asarray(hidden_states, dtype=np.float32)
    cos = np.asarray(cos, dtype=np.float32)
    sin = np.asarray(sin, dtype=np.float32)
    wq_a = np.asarray(wq_a, dtype=np.float32)
    q_a_ln_w = np.asarray(q_a_ln_w, dtype=np.float32)
    wq_b = np.asarray(wq_b, dtype=np.float32)
    wkv_a = np.asarray(wkv_a, dtype=np.float32)
    kv_a_ln_w = np.asarray(kv_a_ln_w, dtype=np.float32)
    wkv_b = np.asarray(wkv_b, dtype=np.float32)
    wo = np.asarray(wo, dtype=np.float32)
    cp = np.asarray(cache_position).astype(np.int64)

    # ---- host-side prep (layout/sharding only) ----
    h = hidden_states[0]                       # [S, D]
    hT = np.ascontiguousarray(h.T).astype(BF)  # [D, S]
    cos_sel = cos[0][cp]                       # [S, DR]
    sin_sel = sin[0][cp]
    cosT = np.ascontiguousarray(cos_sel.T)     # [DR, S]
    sinT = np.ascontiguousarray(sin_sel.T)
    # fold the rmsnorm elementwise weights into the b-projections
    wqb_eff = wq_b * q_a_ln_w[:, None]
    wqb_r3 = wqb_eff.reshape(QR, H, DQK)
    wqbn = np.ascontiguousarray(
        wqb_r3[:, :, :DN].reshape(QR, H * DN)).astype(BF)
    wqbr = np.ascontiguousarray(
        wqb_r3[:, :, DN:].reshape(QR, H * DR)).astype(BF)
    wkvb_eff = wkv_b * kv_a_ln_w[:, None]      # [KVR, KV*(DN+DV)]
    wkvb_r = wkvb_eff.reshape(KVR, KV, DN + DV)
    wkvbk = np.ascontiguousarray(
        wkvb_r[:, :, :DN].reshape(KVR, KV * DN)).astype(BF)
    wkvbv = np.ascontiguousarray(
        wkvb_r[:, :, DN:].reshape(KVR, KV * DV)).astype(BF)
    wo_c = np.ascontiguousarray(wo).astype(BF)
    wqa_c = np.ascontiguousarray(wq_a).astype(BF)
    wkva_c = np.ascontiguousarray(wkv_a).astype(BF)

    sgn = np.concatenate([-np.ones(DR // 2), np.ones(DR // 2)]
                         ).astype(np.float32)[:, None]
    cossinT = np.ascontiguousarray(
        np.concatenate([cosT, sinT * sgn], axis=0))
    in_maps = []
    for c in range(NC_):
        toks = np.arange(c, S, NC_)            # this core's 256 tokens
        hTq = np.ascontiguousarray(hT[:, toks])
        cq = cosT[:, toks]
        sq = (sinT * sgn)[:, toks]
        cosq2 = np.ascontiguousarray(np.concatenate([cq, cq], axis=0))
        sinq2 = np.ascontiguousarray(np.concatenate([sq, sq], axis=0))
        # causal band mask: band[k, j] = 0 if k <= 8j + c else NEG
        k_ = np.arange(128)[:, None]
        j_ = np.arange(16)[None, :]
        band_dev = np.ascontiguousarray(
            np.where(k_ <= 8 * j_ + c, 0.0, NEG).astype(np.float32))
        in_maps.append({
            "hT": hT, "hTq": hTq, "wqa": wqa_c,
            "wqbn": wqbn, "wqbr": wqbr,
            "wkva": wkva_c, "wkvbk": wkvbk, "wkvbv": wkvbv, "wo": wo_c,
            "cossinT": cossinT, "cosq2": cosq2, "sinq2": sinq2,
            "band": band_dev,
        })

    if _BUILT is None:
        _BUILT = _build()
    nc = _BUILT

    res = run_bass_kernel_spmd(nc, in_maps, core_ids=list(range(NC_)),
                               trace=_trace)
    LAST_RESULTS = res

    out_full = np.empty((S, D), dtype=np.float32)
    for c in range(NC_):
        out_full[c::NC_] = res.results[c]["out"]   # row m <-> token 8m+c
    return out_full[None]                      # [1, S, D]
